# revision 1
# baseline (speedup 1.0000x reference)
"""Trainium2 Bass kernel for AttentionPooling with top-k masking.

Math (per graph b, with mask all-ones and N_nodes == N == 1024):
  l      = x @ W^T                      (1024,)   logits
  alpha  = exp(l) / (sum(exp(l)) + 1e-7)
  keep   = rank(l) >= 205  (rank = # of strictly-smaller logits; drops the
           205 smallest-alpha nodes, keeps K = 819)
  idx    = sorted kept indices (ascending)
  x_out  = x[idx] * alpha[idx,None] * 1024     (819, 256)
  A_out  = A[idx][:, idx]                      (819, 819)
  outputs: (x_out, A_out, mask_kept=ones(819,bool), alpha)

Device mapping: data-parallel over batch, 4 graphs per NeuronCore, 8 cores.
Per graph on-device:
  - load x -> logits on DVE (mul+reduce vs broadcast W)
  - replicate logits row to all partitions (DVE 32x32 transpose + small DMAs
    + gpsimd partition_broadcast)
  - rank via 8 DVE tensor_scalar(is_lt, accum) passes against the replicated
    row; keep-mask; v[n] = keep ? n : -1
  - gpsimd sparse_gather compacts v -> ascending kept indices
  - gpsimd dma_gather pulls kept rows of A (4KB rows) and x (1KB rows)
  - gpsimd ap_gather compacts kept columns of the gathered A rows in SBUF
  - alpha / sum(exp) via ACT accumulate on the replicated row
  - x rows rescaled with recomputed kept logits; everything DMAed out
"""

import os
import numpy as np

B, N, C = 32, 1024, 256
K = 819                 # kept rows/cols per graph
NR = 205                # dropped rows
NCORES = 8
G = B // NCORES         # graphs per core
NBLK = 7                # ceil(K/128) row blocks of the gather layout
PADI = 896              # row-gather slots (= NBLK*128, all valid via clamp)
IDXF = PADI // 16       # 56  (wrapped-16 free width of the row index table)
CW = 832                # compacted column slots per row block (52*16 >= 819)
CIDXF = CW // 16        # 52
COMB_N = NBLK * CW      # 5824 column-gather indices per graph
COMB_F = COMB_N // 16   # 364

_CACHE: dict = {}


def _build():
    import concourse.bacc as bacc
    import concourse.tile as tile
    import concourse.mybir as mybir

    f32 = mybir.dt.float32
    i32 = mybir.dt.int32
    i16 = mybir.dt.int16
    u32 = mybir.dt.uint32
    Alu = mybir.AluOpType
    Act = mybir.ActivationFunctionType

    nc = bacc.Bacc("TRN2", target_bir_lowering=False, debug=False,
                   num_devices=NCORES)

    x_in = nc.dram_tensor("x_in", [G, N, C], f32, kind="ExternalInput")
    a_in = nc.dram_tensor("a_in", [G, N, N], f32, kind="ExternalInput")
    w_in = nc.dram_tensor("w_in", [1, C], f32, kind="ExternalInput")
    x_out = nc.dram_tensor("x_out", [G, K, C], f32, kind="ExternalOutput")
    a_out = nc.dram_tensor("a_out", [G, K, K], f32, kind="ExternalOutput")
    al_out = nc.dram_tensor("al_out", [G, N], f32, kind="ExternalOutput")

    with tile.TileContext(nc) as tc:
        with (
            tc.tile_pool(name="const", bufs=1) as pc,
            tc.tile_pool(name="px", bufs=2) as px,
            tc.tile_pool(name="pa", bufs=2) as pa,
            tc.tile_pool(name="pac", bufs=2) as pac,
            tc.tile_pool(name="pxk", bufs=2) as pxk,
            tc.tile_pool(name="ptmp", bufs=2) as ptmp,
            tc.tile_pool(name="pr", bufs=2) as pr,
            tc.tile_pool(name="pjunk", bufs=1) as pjunk,
            tc.tile_pool(name="psm", bufs=2) as psm,
        ):
            # --- constants (once) ---
            w_row = pc.tile([1, C], f32)
            nc.sync.dma_start(w_row[:], w_in[:])
            w_b = pc.tile([128, 1, C], f32)
            nc.gpsimd.partition_broadcast(w_b[:, 0, :], w_row[:])
            iota_i = pc.tile([128, 8], i32)
            # value(p, t) = 1 + p + 128*t  (== node index + 1)
            nc.gpsimd.iota(iota_i[:], pattern=[[128, 8]], base=1,
                           channel_multiplier=1)
            iota1f = pc.tile([128, 8], f32)
            nc.vector.tensor_copy(iota1f[:], iota_i[:])

            junk = pjunk.tile([128, N], f32)    # DVE rank-pass dump
            junk_a = pjunk.tile([128, N], f32)  # ACT exp-sum dump

            for g in range(G):
                # ---- logits: L[p, t] = sum_c x[t*128+p, c] * W[c] ----
                x_sb = px.tile([128, 8, C], f32, tag="xsb")
                nc.sync.dma_start(
                    x_sb[:], x_in[g].rearrange("(t p) c -> p t c", p=128))
                nc.vector.tensor_tensor(
                    out=x_sb[:], in0=x_sb[:],
                    in1=w_b[:].to_broadcast((128, 8, C)), op=Alu.mult)
                L32 = psm.tile([128, 32], f32, tag="L32")
                nc.vector.memset(L32[:, 8:32], 0.0)
                nc.vector.reduce_sum(out=L32[:, 0:8], in_=x_sb[:],
                                     axis=mybir.AxisListType.X)

                # ---- replicate logits into a [1, N] row, then all parts ----
                T32 = psm.tile([128, 32], f32, tag="T32")
                nc.vector.transpose(T32[:], L32[:])
                l_row = psm.tile([1, N], f32, tag="lrow")
                l_row_v = l_row[:].rearrange("o (j i k) -> o j i k", j=8, i=4)
                for i in range(4):
                    nc.sync.dma_start(l_row_v[:, :, i, :],
                                      T32[32 * i:32 * i + 8, 0:32])
                R = pr.tile([128, N], f32, tag="R")
                nc.gpsimd.partition_broadcast(R[:], l_row[:])

                # ---- rank + keep mask + v = keep ? n : -1 ----
                S = psm.tile([128, 8], f32, tag="S")
                for t in range(8):
                    nc.vector.tensor_scalar(
                        out=junk[:], in0=R[:], scalar1=L32[:, t:t + 1],
                        scalar2=None, op0=Alu.is_lt, op1=Alu.add,
                        accum_out=S[:, t:t + 1])
                m01 = psm.tile([128, 8], f32, tag="m01")
                nc.vector.tensor_scalar(
                    out=m01[:], in0=S[:], scalar1=float(NR) - 0.5,
                    scalar2=None, op0=Alu.is_ge)
                v = psm.tile([128, 8], f32, tag="v")
                nc.vector.tensor_tensor(out=v[:], in0=m01[:], in1=iota1f[:],
                                        op=Alu.mult)
                nc.vector.tensor_scalar_add(v[:], v[:], -1.0)

                # ---- wrap v into [16, 64] minor-16 layout ----
                v_w = psm.tile([16, 64], f32, tag="vw")
                v_wv = v_w[:].rearrange("r (t q) -> r t q", q=8)
                for q in range(8):
                    nc.sync.dma_start(v_wv[:, :, q], v[16 * q:16 * q + 16, :])

                # ---- compact kept indices (ascending), pad -1 -> clamp 0 ----
                idx_w = psm.tile([16, IDXF], f32, tag="idxw")
                nf = psm.tile([1, 1], u32, tag="nf")
                nc.gpsimd.sparse_gather(out=idx_w[:], in_=v_w[:],
                                        num_found=nf[:])
                idxc = psm.tile([16, IDXF], f32, tag="idxc")
                nc.vector.tensor_scalar_max(idxc[:], idx_w[:], 0.0)
                idx16 = psm.tile([128, IDXF], i16, tag="idx16")
                nc.vector.tensor_copy(idx16[0:16, :], idxc[:])
                for q in range(1, 8):
                    nc.sync.dma_start(idx16[16 * q:16 * q + 16, :],
                                      idx16[0:16, :])

                # column-gather table: block t slot j -> 1024*t + idx[j]
                comb_f = psm.tile([16, COMB_F], f32, tag="combf")
                for t in range(NBLK):
                    nc.vector.tensor_scalar_add(
                        comb_f[:, CIDXF * t:CIDXF * (t + 1)],
                        idxc[:, 0:CIDXF], float(N * t))
                comb16 = psm.tile([128, COMB_F], i16, tag="comb16")
                nc.vector.tensor_copy(comb16[0:16, :], comb_f[:])
                for q in range(1, 8):
                    nc.sync.dma_start(comb16[16 * q:16 * q + 16, :],
                                      comb16[0:16, :])

                # ---- gather kept rows of A and x from HBM ----
                A_sb = pa.tile([128, NBLK * N], f32, tag="asb")
                nc.gpsimd.dma_gather(
                    out_ap=A_sb[:].rearrange("p (t c) -> p t c", c=N),
                    in_ap=a_in[g], idxs_ap=idx16[:], num_idxs=PADI,
                    num_idxs_reg=PADI, elem_size=N)
                x_kept = pxk.tile([128, NBLK, C], f32, tag="xk")
                nc.gpsimd.dma_gather(
                    out_ap=x_kept[:], in_ap=x_in[g], idxs_ap=idx16[:],
                    num_idxs=PADI, num_idxs_reg=PADI, elem_size=C)

                # ---- alpha normalization pieces ----
                s_all = psm.tile([128, 1], f32, tag="sall")
                nc.scalar.activation(out=junk_a[:], in_=R[:], func=Act.Exp,
                                     accum_out=s_all[:])
                nc.vector.tensor_scalar_add(s_all[:], s_all[:], 1e-7)
                inv = psm.tile([128, 1], f32, tag="inv")
                nc.vector.reciprocal(inv[:], s_all[:])

                # full alpha row -> output
                e_row = psm.tile([1, N], f32, tag="erow")
                nc.scalar.activation(out=e_row[:], in_=l_row[:], func=Act.Exp)
                nc.vector.tensor_scalar(
                    out=e_row[:], in0=e_row[:], scalar1=inv[0:1, 0:1],
                    scalar2=None, op0=Alu.mult)
                nc.sync.dma_start(al_out[g:g + 1, :], e_row[:])

                # ---- x_out = x_kept * exp(l_kept) * inv * N ----
                tmp = ptmp.tile([128, NBLK, C], f32, tag="tmp")
                nc.vector.tensor_tensor(
                    out=tmp[:], in0=x_kept[:],
                    in1=w_b[:].to_broadcast((128, NBLK, C)), op=Alu.mult)
                L_kept = psm.tile([128, NBLK], f32, tag="lk")
                nc.vector.reduce_sum(out=L_kept[:], in_=tmp[:],
                                     axis=mybir.AxisListType.X)
                e_kept = psm.tile([128, NBLK], f32, tag="ek")
                nc.scalar.activation(out=e_kept[:], in_=L_kept[:],
                                     func=Act.Exp)
                scale_k = psm.tile([128, NBLK], f32, tag="sk")
                nc.vector.tensor_scalar(
                    out=scale_k[:], in0=e_kept[:], scalar1=inv[:, 0:1],
                    scalar2=float(N), op0=Alu.mult, op1=Alu.mult)
                nc.vector.tensor_tensor(
                    out=x_kept[:], in0=x_kept[:],
                    in1=scale_k[:].to_broadcast((128, NBLK, C)), op=Alu.mult)
                nc.sync.dma_start(
                    x_out[g, 0:768, :].rearrange("(t p) c -> p t c", p=128),
                    x_kept[:, 0:6, :])
                nc.sync.dma_start(x_out[g, 768:K, :], x_kept[0:51, 6, :])

                # ---- compact kept columns of gathered A rows ----
                A_cmp = pac.tile([128, COMB_N], f32, tag="acmp")
                nc.gpsimd.ap_gather(
                    out_ap=A_cmp[:], in_ap=A_sb[:], idxs_ap=comb16[:],
                    channels=128, num_elems=NBLK * N, d=1, num_idxs=COMB_N)
                A_cmp_v = A_cmp[:].rearrange("p (t j) -> p t j", j=CW)
                nc.sync.dma_start(
                    a_out[g, 0:768, :].rearrange("(t p) j -> p t j", p=128),
                    A_cmp_v[:, 0:6, 0:K])
                nc.sync.dma_start(a_out[g, 768:K, :], A_cmp_v[0:51, 6, 0:K])

    nc.compile()
    return nc


def _get_nc():
    if "nc" not in _CACHE:
        _CACHE["nc"] = _build()
    return _CACHE["nc"]


def _run(x, A, W, trace=False):
    from concourse.bass_utils import run_bass_kernel_spmd

    nc = _get_nc()
    x = np.ascontiguousarray(x, dtype=np.float32)
    A = np.ascontiguousarray(A, dtype=np.float32)
    W = np.ascontiguousarray(W, dtype=np.float32)
    in_maps = [
        {"x_in": x[G * c:G * (c + 1)], "a_in": A[G * c:G * (c + 1)],
         "w_in": W}
        for c in range(NCORES)
    ]
    res = run_bass_kernel_spmd(nc, in_maps, core_ids=list(range(NCORES)),
                               trace=trace)
    x_o = np.concatenate([res.results[c]["x_out"] for c in range(NCORES)])
    a_o = np.concatenate([res.results[c]["a_out"] for c in range(NCORES)])
    al_o = np.concatenate([res.results[c]["al_out"] for c in range(NCORES)])
    return (x_o, a_o, np.ones((B, K), dtype=bool), al_o), res


def kernel(x, A, W, mask, N_nodes):
    outs, _ = _run(x, A, W, trace=False)
    return outs
